# revision 15
# baseline (speedup 1.0000x reference)
"""
Trainium2 distributed kernel for causal multi-head attention
(nn_Attention: B=4, S=2048, D=768, H=4, DH=192).

Sharding: 16 (batch, head) units across 8 cores = 1 batch x 2 heads per
core.  Every core runs an identical graph (SPMD) on its own shard; the
host sums core pairs (the unshard for output-partial sharding).  No
on-device collectives, perfectly balanced causal work.

Device algorithm (bf16 matmuls, f32 PSUM accumulation):
  QT/KT stored transposed [head-dim planes, seq]; the two heads' upper
  64 head-dims share one 128-partition plane (host permutes weight
  columns to match), so every projection matmul contracts a full 128
  partitions and the two 64-row score matmuls run in disjoint PE row
  groups (concurrent).  V is stored naturally [seq, dh] with per-head
  ones columns so the AV matmul also emits softmax denominators.
  Scores are computed transposed, S.T[k, q] = KT.T @ QT, so exp'd
  attention tiles feed AV directly as the moving operand — no
  transposes anywhere.  Softmax skips max-subtraction (logits are O(1)
  by construction); causality is applied post-exp as a multiplicative
  0/1 bf16 mask on the diagonal blocks only (fast DVE mode, off the
  PSUM->exp chain); fully-masked blocks are never computed.

  Per q-block the AV runs the denominator-carrying 65-row psum chains
  for both heads FIRST, so the softmax denominators are ready while the
  main 128-row AV chains still run; the reciprocal is exp(-ln(d)) on
  ScalarE (both functions live in one activation table set), which is
  ~5x faster than the DVE's iterative-divide RECIPROCAL and sits on a
  different engine than the normalize multiplies.  The denominator
  broadcast matmul is issued after the main AV chains so it never
  head-of-line-blocks the PE queue.  Big q-blocks are processed first
  and the deferred output projection threads between score and AV
  phases so PE never waits on the ScalarE exp tails.  Output is
  emitted bf16 (host accumulates the two half-head partials in f32),
  halving the output DMA.  A burst of dummy matmuls during the input
  DMA lead-in pre-warms the PE HAM clock gate.
"""

import math
import os
import sys

import numpy as np

for _p in ("/opt/trn_rl_repo",):
    if _p not in sys.path and os.path.isdir(_p):
        sys.path.insert(0, _p)

import ml_dtypes  # noqa: E402

B, S, D, H = 4, 2048, 768, 4
DH = D // H  # 192
HPC = 2  # heads per core
HD = HPC * DH  # 384 head dims per core
P = 128
KD = D // P  # 6 contraction chunks over D
QB = 512  # query block (matmul moving dim)
NQ = S // QB  # 4
KB = 128  # key block (psum partition dim)
NK = S // KB  # 16
MS = S // P  # 16 seq chunks
SCALE = 1.0 / math.sqrt(DH)
MASK_NEG = -1e9

# host-side column permutation for Wq/Wk (and row perm for Wo):
# planes = [h0 dh0:128 | h1 dh0:128 | h0 dh128:192, h1 dh128:192]
PQ = np.r_[0:128, 192:320, 128:192, 320:384]
# for Wv: [h0 dh0:192 | h1 dh128:192 | h1 dh0:128] so that the SBUF V
# tile [.. h0dh(192), ones0, ones1, h1dh128:192(64), h1dh0:128(128)]
# fills with two contiguous copies
PV = np.r_[0:192, 320:384, 192:320]

_CACHED = {}


def build_nc(reps=1, use_pool=False, sc2=False, actcopy=True):
    import concourse.mybir as mybir
    from concourse import bacc
    from concourse import tile

    fp32 = mybir.dt.float32
    bf16 = mybir.dt.bfloat16
    Exp = mybir.ActivationFunctionType.Exp
    Ln = mybir.ActivationFunctionType.Ln

    # The act-table placement pass loads, for each activation whose
    # function may be missing from the currently-loaded table set, the
    # first act_info set containing it.  'natural_log' precedes
    # 'natural_log_exp_and_others', so mixing Exp and Ln ping-pongs
    # between two sets (~1.5us ACT_TABLE_LOAD each, on the exp-stream
    # critical path).  Patch the python-side placement map (set order
    # and indices unchanged, so the emitted act_func_set_id still
    # matches act_info.json) to advertise exp/ln ONLY in the combined
    # set: placement then settles on one load.  Runtime table contents
    # are untouched.
    from concourse import hw_specs as _hw

    if not hasattr(bacc, "_orig_get_activation_tables"):
        bacc._orig_get_activation_tables = bacc.get_activation_tables

        def _patched_tables(arch):
            both = {Exp, Ln}
            out = {}
            for name, fns in bacc._orig_get_activation_tables(arch).items():
                if name != "natural_log_exp_and_others":
                    fns = set(fns) - both
                out[name] = fns
            return out

        bacc.get_activation_tables = _patched_tables
        _hw.get_activation_tables = _patched_tables

    nc = bacc.Bacc(None, target_bir_lowering=False)

    xT = nc.declare_dram_parameter("xT", [D, S], bf16, isOutput=False)
    wqT = nc.declare_dram_parameter("wqT", [D, HD], bf16, isOutput=False)
    wkT = nc.declare_dram_parameter("wkT", [D, HD], bf16, isOutput=False)
    wvT = nc.declare_dram_parameter("wvT", [D, HD], bf16, isOutput=False)
    woS = nc.declare_dram_parameter("woS", [HD, D], bf16, isOutput=False)
    out = nc.declare_dram_parameter("out", [S, D], bf16, isOutput=True)

    # V sbuf free-layout offsets
    V_H0C0 = slice(0, 128)
    V_H0C1 = slice(128, 193)  # h0 dh128:192 + ones0 @192 -> denom row 64
    V_H1C1 = slice(193, 258)  # h1 dh128:192 + ones1 @257 -> denom row 64
    V_H1C0 = slice(258, 386)
    VW = 386

    with tile.TileContext(nc) as tc:
        with (
            tc.tile_pool(name="const", bufs=1) as const,
            tc.tile_pool(name="atp", bufs=2) as atp,
            tc.tile_pool(name="ost", bufs=3) as ostp,
            tc.tile_pool(name="rcp", bufs=2) as rcp,
            tc.tile_pool(name="scps", bufs=2, space="PSUM") as scps,
            tc.tile_pool(name="avps", bufs=1, space="PSUM") as avps,
        ):
            # ---- persistent SBUF tensors ----
            xT_sb = const.tile([P, KD, S], bf16, tag="xT_sb")
            wq_sb = const.tile([P, KD, HD], bf16, tag="wq_sb")
            wk_sb = const.tile([P, KD, HD], bf16, tag="wk_sb")
            wv_sb = const.tile([P, KD, HD], bf16, tag="wv_sb")
            wo_sb = const.tile([P, 3, D], bf16, tag="wo_sb")
            qt_sb = const.tile([P, 3, S], bf16, tag="qt_sb")
            kt_sb = const.tile([P, 3, S], bf16, tag="kt_sb")
            v_sb = const.tile([P, NK, VW], bf16, tag="v_sb")
            pt_sb = const.tile([P, 3, S], bf16, tag="pt_sb")
            ones1 = const.tile([1, P], bf16, tag="ones1")
            warm = const.tile([1, 2], fp32, tag="warm")
            wmm = const.tile([P, QB], bf16, tag="wmm")
            masks = const.tile([P, 4, 2 * QB], bf16, tag="masks")

            # ---- input DMAs ----
            # priority order per queue: everything the projection waves
            # consume first (xT planes, wq, wk), wv/wo demoted behind
            # them — they are only needed tens of microseconds in, and
            # the DMA fabric is bandwidth-saturated during the lead-in.
            for k in range(KD):
                nc.scalar.dma_start(
                    wq_sb[:, k, :], wqT[k * P : (k + 1) * P, :]
                )
                nc.gpsimd.dma_start(
                    wk_sb[:, k, :], wkT[k * P : (k + 1) * P, :]
                )
                nc.sync.dma_start(
                    xT_sb[:, k, :], xT[k * P : (k + 1) * P, :]
                )
            # wv/wo ride the sync queue BEHIND the xT planes: the DMA
            # fabric is saturated during the lead-in and these 1.7MB are
            # not needed until tens of microseconds in — the per-queue
            # descriptor ring serializes them after the planes.
            nc.sync.dma_start(
                wv_sb[:], wvT.rearrange("(ko ki) j -> ki ko j", ki=P)
            )
            for c in range(3):
                nc.sync.dma_start(
                    wo_sb[:, c, :], woS[c * P : (c + 1) * P, :]
                )

            # PE HAM warm-up: the first ~7us are DMA lead-in with no
            # matmul work; a burst of dummy matmuls on a memset tile
            # flips the clock gate to full rate before real data lands.
            nc.vector.memset(wmm[:], 0.0)
            wps = scps.tile([P, QB], fp32, tag="sc", name="warm_ps")
            for _ in range(12):
                nc.tensor.matmul(
                    wps, lhsT=wmm[:, 0:P], rhs=wmm[:],
                    start=True, stop=True,
                )

            nc.vector.memset(ones1[:], 1.0)
            # prefetch the ln+exp table while the PE does projections
            # (Ln first so the activation-table pass settles on the
            # natural_log_exp set, which contains both)
            nc.scalar.activation(warm[:, 0:1], ones1[0:1, 0:1], Ln)
            nc.scalar.activation(warm[:, 1:2], ones1[0:1, 0:1], Exp)
            # ones columns of V are static: set them once
            nc.vector.memset(v_sb[:, :, 192:193], 1.0)
            nc.vector.memset(v_sb[:, :, 257:258], 1.0)

            # multiplicative causal masks (0/1 bf16) for the 4 diagonal
            # sub-blocks, double width to cover both heads' fused at tile:
            # keep 1 iff q_local >= 128*d + k_local, else 0
            for d in range(4):
                nc.vector.memset(masks[:, d, :], 1.0)
                nc.gpsimd.affine_select(
                    out=masks[:, d, :],
                    in_=masks[:, d, :],
                    compare_op=mybir.AluOpType.is_ge,
                    fill=0.0,
                    base=-128 * d,
                    pattern=[[0, 2], [1, QB]],
                    channel_multiplier=-1,
                )

            # ---- Q/K projections (transposed outputs, 3 full planes) ----
            def wide_wave(c):
                # per xT plane: Q-c's 4 groups (av tags) + K-c's first 2
                # groups (sc slots) -> ~6 matmuls per plane arrival; the
                # k-outer order lets plane-0..3 work run before the last
                # planes land from DRAM
                pssQ = [
                    avps.tile([P, QB], fp32, tag=t, name=f"wwq{c}_{t}")
                    for t in ("avA", "avB", "avC", "avD")
                ]
                pssK = [
                    scps.tile([P, QB], fp32, tag="sc", name=f"wwk{c}_{i}")
                    for i in range(2)
                ]
                csl = slice(c * P, (c + 1) * P)
                for k in range(KD):
                    for nt in range(NQ):
                        nc.tensor.matmul(
                            pssQ[nt],
                            lhsT=wq_sb[:, k, csl],
                            rhs=xT_sb[:, k, nt * QB : (nt + 1) * QB],
                            start=(k == 0), stop=(k == KD - 1),
                        )
                    for nt in range(2):
                        nc.tensor.matmul(
                            pssK[nt],
                            lhsT=wk_sb[:, k, csl],
                            rhs=xT_sb[:, k, nt * QB : (nt + 1) * QB],
                            start=(k == 0), stop=(k == KD - 1),
                        )
                for nt in range(NQ):
                    (nc.scalar.copy if actcopy else nc.vector.tensor_copy)(
                        qt_sb[:, c, nt * QB : (nt + 1) * QB], pssQ[nt]
                    )
                for nt in range(2):
                    (nc.scalar.copy if actcopy else nc.vector.tensor_copy)(
                        kt_sb[:, c, nt * QB : (nt + 1) * QB], pssK[nt]
                    )
                # K-c's remaining 2 groups (planes all resident by now)
                for nt in (2, 3):
                    ps = avps.tile(
                        [P, QB], fp32, tag="av" + "ABCD"[nt], name=f"kc{c}{nt}"
                    )
                    for k in range(KD):
                        nc.tensor.matmul(
                            ps,
                            lhsT=wk_sb[:, k, csl],
                            rhs=xT_sb[:, k, nt * QB : (nt + 1) * QB],
                            start=(k == 0), stop=(k == KD - 1),
                        )
                    (nc.scalar.copy if actcopy else nc.vector.tensor_copy)(
                        kt_sb[:, c, nt * QB : (nt + 1) * QB], ps
                    )

            def projections(first=False):
                if first:
                    wide_wave(0)
                for w_sb, o_sb in ((wq_sb, qt_sb), (wk_sb, kt_sb)):
                    for c in range(1 if first else 0, 3):
                        for nt in range(NQ):
                            ps = avps.tile(
                                [P, QB], fp32,
                                tag="av" + "ABCD"[nt], name=f"pj{c}{nt}",
                            )
                            for k in range(KD):
                                nc.tensor.matmul(
                                    ps,
                                    lhsT=w_sb[:, k, c * P : (c + 1) * P],
                                    rhs=xT_sb[:, k, nt * QB : (nt + 1) * QB],
                                    start=(k == 0),
                                    stop=(k == KD - 1),
                                )
                            (nc.scalar.copy if actcopy
                             else nc.vector.tensor_copy)(
                                o_sb[:, c, nt * QB : (nt + 1) * QB], ps
                            )
                # ---- V projection (natural layout) + ones columns ----
                for m in range(MS):
                    ps = avps.tile(
                        [P, QB], fp32, tag="av" + "ABCD"[m % 4], name=f"pv{m}"
                    )
                    for k in range(KD):
                        nc.tensor.matmul(
                            ps[:, 0:HD],
                            lhsT=xT_sb[:, k, m * P : (m + 1) * P],
                            rhs=wv_sb[:, k, :],
                            start=(k == 0),
                            stop=(k == KD - 1),
                        )
                    (nc.scalar.copy if actcopy else nc.vector.tensor_copy)(
                        v_sb[:, m, 0:192], ps[:, 0:192]
                    )
                    nc.vector.tensor_copy(v_sb[:, m, 193:257], ps[:, 192:256])
                    nc.vector.tensor_copy(v_sb[:, m, 258:386], ps[:, 256:384])

            # ---- attention per q-block; out-proj deferred one block ----
            # out DMAs ride sync/gpsimd only: a DMA trigger costs
            # ~0.7us of NX time, which on the scalar queue would wedge
            # between exps of the attention stream.
            def out_proj(qj, mis=(0, 1, 2, 3), on_act=False, split=False):
                for mi in mis:
                    m = qj * 4 + mi
                    ost = ostp.tile([P, D], bf16, tag="ost")
                    for n in range(2):
                        ps = scps.tile(
                            [P, QB], fp32, tag="sc", name=f"op{mi}{n}"
                        )
                        for c in range(3):
                            nc.tensor.matmul(
                                ps[:, 0:384],
                                lhsT=pt_sb[:, c, m * P : (m + 1) * P],
                                rhs=wo_sb[:, c, n * 384 : (n + 1) * 384],
                                start=(c == 0),
                                stop=(c == 2),
                            )
                        if split:
                            # tail blocks: alternate evacuation engines
                            # (psum-slot reuse then gates on two engines
                            # instead of one) and DMA each half as soon
                            # as its copy lands — shortens the final
                            # copy->DMA drain chain
                            (nc.scalar.copy if n == 0
                             else nc.vector.tensor_copy)(
                                ost[:, n * 384 : (n + 1) * 384], ps[:, 0:384]
                            )
                            [nc.sync, nc.gpsimd][(2 * m + n) % 2].dma_start(
                                out[m * P : (m + 1) * P,
                                    n * 384 : (n + 1) * 384],
                                ost[:, n * 384 : (n + 1) * 384],
                            )
                        else:
                            (nc.scalar.copy if on_act
                             else nc.vector.tensor_copy)(
                                ost[:, n * 384 : (n + 1) * 384], ps[:, 0:384]
                            )
                    if not split:
                        [nc.sync, nc.gpsimd][m % 2].dma_start(
                            out[m * P : (m + 1) * P, :], ost[:]
                        )

            def scores_part(qj):
                qsl = slice(qj * QB, (qj + 1) * QB)
                nk = 4 * qj + 4  # live key blocks (causal)
                # fused at tile: both heads side by side [.., h0 512 | h1 512]
                at2 = atp.tile(
                    [P, NK, 2 * QB], bf16, tag="at2", name=f"at2_{qj}"
                )
                for ki in range(nk):
                    ksl = slice(ki * KB, (ki + 1) * KB)
                    # one 2-bank psum tile holds both heads' score block
                    ps = scps.tile([P, 2 * QB], fp32, tag="sc")
                    ps0 = ps[:, 0:QB]
                    ps1 = ps[:, QB : 2 * QB]
                    # full-plane matmuls (128 contraction rows)
                    nc.tensor.matmul(
                        ps0, lhsT=kt_sb[:, 0, ksl], rhs=qt_sb[:, 0, qsl],
                        start=True, stop=False,
                    )
                    nc.tensor.matmul(
                        ps1, lhsT=kt_sb[:, 1, ksl], rhs=qt_sb[:, 1, qsl],
                        start=True, stop=False,
                    )
                    # 64-row tails in disjoint row groups (concurrent)
                    nc.tensor.matmul(
                        ps0, lhsT=kt_sb[0:64, 2, ksl], rhs=qt_sb[0:64, 2, qsl],
                        start=False, stop=True,
                    )
                    nc.tensor.matmul(
                        ps1,
                        lhsT=kt_sb[64:128, 2, ksl],
                        rhs=qt_sb[64:128, 2, qsl],
                        start=False, stop=True,
                    )
                    d = ki - 4 * qj
                    # one exp for both heads: amortizes the ACT ramp
                    nc.scalar.activation(at2[:, ki, :], ps, Exp, scale=SCALE)
                    if d >= 0:
                        # multiplicative causal zeroing post-exp: bf16 SBUF
                        # DVE fast mode, off the PSUM->exp chain
                        nc.vector.tensor_mul(
                            at2[:, ki, :], at2[:, ki, :], masks[:, d, :]
                        )
                return at2

            def av_part(qj, at2, mid=None):
                qsl = slice(qj * QB, (qj + 1) * QB)
                nk = 4 * qj + 4
                at0 = at2[:, :, 0:QB]
                at1 = at2[:, :, QB : 2 * QB]

                # 65-row chains (upper 64 head dims + ones denom row)
                # FIRST for both heads, so the denominators complete
                # early and the reciprocal overlaps the main AV chains
                psc1_0 = avps.tile([P, QB], fp32, tag="avC", name=f"av1_{qj}0")
                psc1_1 = avps.tile([P, QB], fp32, tag="avD", name=f"av1_{qj}1")
                for ki in range(nk):
                    nc.tensor.matmul(
                        psc1_0[0:65], lhsT=v_sb[:, ki, V_H0C1],
                        rhs=at0[:, ki, :],
                        start=(ki == 0), stop=(ki == nk - 1),
                    )
                    nc.tensor.matmul(
                        psc1_1[0:65], lhsT=v_sb[:, ki, V_H1C1],
                        rhs=at1[:, ki, :],
                        start=(ki == 0), stop=(ki == nk - 1),
                    )
                # reciprocal of the denominators on ScalarE: 1/d =
                # exp(-ln(d)); both functions share one activation
                # table set, and ScalarE streams 1 elem/cycle vs the
                # DVE RECIPROCAL's 1/8 rate
                lnr = rcp.tile([1, 2, QB], fp32, tag="lnr", name=f"ln_{qj}")
                rc2 = rcp.tile([1, 2, QB], bf16, tag="rc2", name=f"rc_{qj}")
                # per-head ln->exp pairs: the first head's broadcast
                # only waits ~1.4us instead of the full fused chain
                nc.scalar.activation(lnr[:, 0], psc1_0[64:65, :], Ln)
                with nc.allow_low_precision(
                    reason="bf16 recip feeds broadcast matmul"
                ):
                    nc.scalar.activation(rc2[:, 0], lnr[:, 0], Exp, scale=-1.0)
                nc.scalar.activation(lnr[:, 1], psc1_1[64:65, :], Ln)
                with nc.allow_low_precision(
                    reason="bf16 recip feeds broadcast matmul"
                ):
                    nc.scalar.activation(rc2[:, 1], lnr[:, 1], Exp, scale=-1.0)

                # main 128-row AV chains (at tiles all exp'd by now)
                psc0_0 = avps.tile([P, QB], fp32, tag="avA", name=f"av0_{qj}0")
                psc0_1 = avps.tile([P, QB], fp32, tag="avB", name=f"av0_{qj}1")
                for ki in range(nk):
                    nc.tensor.matmul(
                        psc0_0, lhsT=v_sb[:, ki, V_H0C0], rhs=at0[:, ki, :],
                        start=(ki == 0), stop=(ki == nk - 1),
                    )
                    nc.tensor.matmul(
                        psc0_1, lhsT=v_sb[:, ki, V_H1C0], rhs=at1[:, ki, :],
                        start=(ki == 0), stop=(ki == nk - 1),
                    )
                if mid is not None:
                    mid()  # independent PE work to cover the recip chain
                # broadcast both heads' reciprocals across partitions;
                # issued AFTER the main chains so the wait on rc2 never
                # head-of-line-blocks the PE queue.  One 2-bank slot so
                # the other score slot stays free for the next block.
                rcb2 = scps.tile(
                    [P, 2 * QB], fp32, tag="sc", name=f"rcb2_{qj}"
                )
                nc.tensor.matmul(
                    rcb2[:, 0:QB], lhsT=ones1[:], rhs=rc2[:, 0, :],
                    start=True, stop=True,
                )
                nc.tensor.matmul(
                    rcb2[:, QB : 2 * QB], lhsT=ones1[:], rhs=rc2[:, 1, :],
                    start=True, stop=True,
                )
                rcb = rcp.tile([P, 2, QB], fp32, tag="rcb", name=f"rcb_{qj}")
                # per-head evacuation: the first pt-plane multiply only
                # waits on its own head's broadcast half
                nc.vector.tensor_copy(rcb[:, 0, :], rcb2[:, 0:QB])
                nc.vector.tensor_mul(pt_sb[:, 0, qsl], psc0_0, rcb[:, 0, :])
                nc.vector.tensor_copy(rcb[:, 1, :], rcb2[:, QB : 2 * QB])
                nc.vector.tensor_mul(pt_sb[:, 1, qsl], psc0_1, rcb[:, 1, :])
                # upper 64 head dims land in plane 2: h0 -> partitions
                # 0:64, h1 -> partitions 64:128 (partition-shifted write)
                nc.vector.tensor_mul(
                    pt_sb[0:64, 2, qsl], psc1_0[0:64], rcb[0:64, 0, :]
                )
                nc.vector.tensor_mul(
                    pt_sb[64:128, 2, qsl], psc1_1[0:64], rcb[64:128, 1, :]
                )

            # big q-blocks first; every AV waits one-block-deferred so the
            # next block's scores cover its exp tail, and out-proj halves
            # cover the norm chains
            for _rep in range(reps):
                projections(first=(_rep == 0))
                a3 = scores_part(3)
                a2 = scores_part(2)
                av_part(3, a3)
                a1 = scores_part(1)
                out_proj(3, (0, 1))
                av_part(2, a2)
                a0 = scores_part(0)
                out_proj(3, (2, 3))
                out_proj(2, (0, 1))
                av_part(1, a1)
                out_proj(2, (2, 3))
                av_part(
                    0, a0,
                    mid=lambda: out_proj(1, (0, 1), on_act=True),
                )
                out_proj(1, (2, 3), split=True)
                out_proj(0, split=True)

    nc.compile()
    return nc


def _shard_inputs(x, Wq, Wk, Wv, Wo):
    bf = ml_dtypes.bfloat16
    in_maps = []
    for core in range(8):
        b, hp = core // 2, core % 2
        cols = slice(hp * HD, (hp + 1) * HD)
        in_maps.append(
            {
                "xT": np.ascontiguousarray(x[b].T).astype(bf),
                "wqT": np.ascontiguousarray(Wq[cols, :].T[:, PQ]).astype(bf),
                "wkT": np.ascontiguousarray(Wk[cols, :].T[:, PQ]).astype(bf),
                "wvT": np.ascontiguousarray(Wv[cols, :].T[:, PV]).astype(bf),
                "woS": np.ascontiguousarray(Wo[:, cols].T[PQ, :]).astype(bf),
            }
        )
    return in_maps


def _run(inputs, trace=False, **kw):
    from concourse.bass_utils import run_bass_kernel_spmd

    if "nc" not in _CACHED:
        _CACHED["nc"] = build_nc()
    nc = _CACHED["nc"]
    in_maps = _shard_inputs(
        np.asarray(inputs["x"], np.float32),
        np.asarray(inputs["Wq"], np.float32),
        np.asarray(inputs["Wk"], np.float32),
        np.asarray(inputs["Wv"], np.float32),
        np.asarray(inputs["Wo"], np.float32),
    )
    res = run_bass_kernel_spmd(
        nc, in_maps, core_ids=list(range(8)), trace=trace, **kw
    )
    parts = [np.asarray(r["out"], np.float32) for r in res.results]
    full = np.empty((B, S, D), np.float32)
    for b in range(B):
        full[b] = parts[2 * b] + parts[2 * b + 1]
    return full, res


def kernel(**inputs) -> np.ndarray:
    full, _ = _run(inputs, trace=False)
    return full
